# revision 6
# baseline (speedup 1.0000x reference)
"""Fused attention-GNN kernel for Trainium2 (8 NeuronCores, data-parallel over batch).

Per batch element b (one per core):
    q = text @ Wq + bq ; k = text @ Wk + bk ; v = text @ Wv + bv
    E^T = exp((k @ q^T) / sqrt(D))          # scores transposed, no max-sub needed
    W = adj @ v
    U_aug = E^T.T @ [adj | 1 | W]           # U, rowsum(E), E@W in one accumulation
    new_adj = U / rowsum ; output = (E@W) / rowsum

All matmuls in bf16 (fp32 accumulation in PSUM); transposes via DMA xbar.
"""

import math
import os
import sys

for _p in ("/opt/trn_rl_repo",):
    if _p not in sys.path:
        sys.path.insert(0, _p)

import numpy as np

import concourse.bass as bass
import concourse.tile as tile
from concourse import bacc, mybir
from concourse.bass_utils import run_bass_kernel_spmd

P = 128
N = 2048
D = 256
NT = N // P  # 16 row chunks
DC = D // P  # 2 contraction chunks
FS = 512  # matmul free-dim slice (one PSUM bank of f32)
NS = N // FS  # 4 slices
TEMP = math.sqrt(D)
CDT = mybir.dt.bfloat16
F32 = mybir.dt.float32

# fp8(e4m3) + DoubleRow for the dominant [N,N]x[N,N] matmul; bf16 elsewhere.
BIG_FP8 = os.environ.get("BIG_FP8", "0") == "1"
BDT = mybir.dt.float8e4 if BIG_FP8 else CDT
OW = 272 if BIG_FP8 else 1 + D  # onesW free size (DoubleRow needs step % 16 == 0)

_CACHE = {}
LAST_RESULTS = None


def _build():
    nc = bacc.Bacc("TRN2", target_bir_lowering=False, debug=False)
    text = nc.dram_tensor("text", [N, D], F32, kind="ExternalInput").ap()
    adj = nc.dram_tensor("adj", [N, N], F32, kind="ExternalInput").ap()
    wq = nc.dram_tensor("Wq", [D, D], F32, kind="ExternalInput").ap()
    bq = nc.dram_tensor("bq", [D], F32, kind="ExternalInput").ap()
    wk = nc.dram_tensor("Wk", [D, D], F32, kind="ExternalInput").ap()
    bk = nc.dram_tensor("bk", [D], F32, kind="ExternalInput").ap()
    wv = nc.dram_tensor("Wv", [D, D], F32, kind="ExternalInput").ap()
    bv = nc.dram_tensor("bv", [D], F32, kind="ExternalInput").ap()
    out_feat = nc.dram_tensor("out_feat", [N, D], F32, kind="ExternalOutput").ap()
    out_adj = nc.dram_tensor("out_adj", [N, N], F32, kind="ExternalOutput").ap()

    text_d = text.rearrange("(nt p) i -> p nt i", p=P)  # [128,16,256]
    adj_d = adj.rearrange("(mt p) k -> p mt k", p=P)  # [128,16,2048]
    wq_d = wq.rearrange("(ic p) o -> p ic o", p=P)  # [128,2,256]
    wk_d = wk.rearrange("(ic p) o -> p ic o", p=P)
    wv_d = wv.rearrange("(ic p) o -> p ic o", p=P)
    bq_d = bq.rearrange("(oc p) -> p oc", p=P)  # [128,2]
    bk_d = bk.rearrange("(oc p) -> p oc", p=P)
    ofeat_d = out_feat.rearrange("(nt p) d -> p nt d", p=P)
    oadj_d = out_adj.rearrange("(nt p) k -> p nt k", p=P)

    with tile.TileContext(nc) as tc:
        with (
            tc.tile_pool(name="persist", bufs=1) as persist,
            tc.tile_pool(name="stage", bufs=1) as stage,
        ):
            # ---- weights / biases ----
            wq_sb = persist.tile([P, DC, D], CDT, name="wq_sb")
            wk_sb = persist.tile([P, DC, D], CDT, name="wk_sb")
            wv_sb = persist.tile([P, DC, D], CDT, name="wv_sb")
            nc.gpsimd.dma_start(out=wq_sb, in_=wq_d)  # f32 -> bf16 cast in DMA
            nc.gpsimd.dma_start(out=wk_sb, in_=wk_d)
            nc.gpsimd.dma_start(out=wv_sb, in_=wv_d)
            bq_sb = persist.tile([P, DC], F32, name="bq_sb")
            bk_sb = persist.tile([P, DC], F32, name="bk_sb")
            nc.sync.dma_start(out=bq_sb, in_=bq_d)
            nc.sync.dma_start(out=bk_sb, in_=bk_d)
            bv_bc = persist.tile([P, D], F32, name="bv_bc")
            bv_src = bass.AP(tensor=bv.tensor, offset=bv.offset, ap=[[0, P]] + list(bv.ap))
            nc.gpsimd.dma_start(out=bv_bc, in_=bv_src)

            v_sb = persist.tile([P, NT, D], CDT, name="v_sb")
            ET = persist.tile([P, NT, N], BDT, name="ET")  # E^T: [m-in-chunk, mc, n]
            adj_sb = persist.tile([P, NT, N], BDT, name="adj_sb")  # [m-in-chunk, mc, k]
            onesW = persist.tile([P, NT, OW], BDT, name="onesW")  # [ones | W]
            nc.vector.memset(onesW[:, :, 0:1], 1.0)

            with (
                tc.tile_pool(name="early", bufs=1) as early,
                tc.tile_pool(name="psE", bufs=1, space="PSUM") as psE,
            ):
                # ---- text -> textT (bf16) ----
                textT = early.tile([P, DC, N], CDT, name="textT")  # [i-in-chunk, ic, n]
                for nt in range(NT):
                    tch = early.tile([P, D], CDT, tag="tch", bufs=2, name=f"tch{nt}")
                    nc.gpsimd.dma_start(out=tch, in_=text_d[:, nt, :])
                    nc.sync.dma_start(
                        out=textT[:, :, nt * P : (nt + 1) * P], in_=tch, transpose=True
                    )

                # ---- qT, kT = (Wq^T stationary) x textT ----
                qT = early.tile([P, DC, N], CDT, name="qT")  # [o-in-chunk, oc, n]
                kT = early.tile([P, DC, N], CDT, name="kT")
                for wsb, bsb, dst, nm in (
                    (wq_sb, bq_sb, qT, "q"),
                    (wk_sb, bk_sb, kT, "k"),
                ):
                    for oc in range(DC):
                        for ns in range(NS):
                            pq = psE.tile(
                                [P, FS], F32, tag="acc", bufs=2, name=f"p{nm}{oc}_{ns}"
                            )
                            for ic in range(DC):
                                nc.tensor.matmul(
                                    pq,
                                    lhsT=wsb[:, ic, oc * P : (oc + 1) * P],
                                    rhs=textT[:, ic, ns * FS : (ns + 1) * FS],
                                    start=(ic == 0),
                                    stop=(ic == DC - 1),
                                )
                            nc.vector.tensor_scalar_add(
                                out=dst[:, oc, ns * FS : (ns + 1) * FS],
                                in0=pq,
                                scalar1=bsb[:, oc : oc + 1],
                            )

                # ---- v (natural layout) = textT-tiles stationary x Wv ----
                for kc in range(NT):
                    pv = psE.tile([P, D], F32, tag="acc", bufs=2, name=f"pv{kc}")
                    for ic in range(DC):
                        nc.tensor.matmul(
                            pv,
                            lhsT=textT[:, ic, kc * P : (kc + 1) * P],
                            rhs=wv_sb[:, ic, :],
                            start=(ic == 0),
                            stop=(ic == DC - 1),
                        )
                    nc.vector.tensor_add(out=v_sb[:, kc, :], in0=pv, in1=bv_bc)

                # ---- scoresT -> exp -> ET ----
                for mc in range(NT):
                    for ns in range(NS):
                        psc = psE.tile(
                            [P, FS], F32, tag="s", bufs=4, name=f"ps{mc}_{ns}"
                        )
                        for oc in range(DC):
                            nc.tensor.matmul(
                                psc,
                                lhsT=kT[:, oc, mc * P : (mc + 1) * P],
                                rhs=qT[:, oc, ns * FS : (ns + 1) * FS],
                                start=(oc == 0),
                                stop=(oc == DC - 1),
                            )
                        nc.scalar.activation(
                            out=ET[:, mc, ns * FS : (ns + 1) * FS],
                            in_=psc,
                            func=mybir.ActivationFunctionType.Exp,
                            scale=1.0 / TEMP,
                        )

                # ---- adj load (cast) + W = adj @ v ----
                for mc in range(NT):
                    if BIG_FP8:
                        abf = early.tile([P, N], CDT, tag="abf", bufs=2, name=f"ab{mc}")
                        nc.gpsimd.dma_start(out=abf, in_=adj_d[:, mc, :])
                        nc.vector.tensor_copy(out=adj_sb[:, mc, :], in_=abf)
                        tsrc = abf
                    else:
                        nc.gpsimd.dma_start(out=adj_sb[:, mc, :], in_=adj_d[:, mc, :])
                        tsrc = adj_sb[:, mc, :]
                    adjT = stage.tile(
                        [P, NT, P], CDT, tag="adjT", bufs=2, name=f"adjT{mc}"
                    )  # [k-in-chunk, kc, m]
                    nc.sync.dma_start(out=adjT, in_=tsrc, transpose=True)
                    pw = psE.tile([P, D], F32, tag="w", bufs=2, name=f"pw{mc}")
                    for kc in range(NT):
                        nc.tensor.matmul(
                            pw,
                            lhsT=adjT[:, kc, :],
                            rhs=v_sb[:, kc, :],
                            start=(kc == 0),
                            stop=(kc == NT - 1),
                        )
                    nc.vector.tensor_copy(out=onesW[:, mc, 1 : 1 + D], in_=pw)

            # ---- big fused matmul: U | rowsum | E@W ----
            with tc.tile_pool(name="psL", bufs=1, space="PSUM") as psL:
                for nch in range(NT):
                    pbig = [
                        psL.tile([P, FS], F32, tag="big", bufs=8, name=f"pb{nch}_{s}")
                        for s in range(NS + 1)
                    ]
                    if BIG_FP8:
                        DR = mybir.MatmulPerfMode.DoubleRow
                        for mp in range(NT // 2):
                            pair = slice(2 * mp, 2 * mp + 2)
                            lhsT = ET[:, pair, nch * P : (nch + 1) * P]
                            for s in range(NS):
                                nc.tensor.matmul(
                                    pbig[s],
                                    lhsT=lhsT,
                                    rhs=adj_sb[:, pair, s * FS : (s + 1) * FS],
                                    start=(mp == 0),
                                    stop=(mp == NT // 2 - 1),
                                    perf_mode=DR,
                                )
                            nc.tensor.matmul(
                                pbig[NS][:, : 1 + D],
                                lhsT=lhsT,
                                rhs=onesW[:, pair, : 1 + D],
                                start=(mp == 0),
                                stop=(mp == NT // 2 - 1),
                                perf_mode=DR,
                            )
                    else:
                        for mc in range(NT):
                            lhsT = ET[:, mc, nch * P : (nch + 1) * P]
                            for s in range(NS):
                                nc.tensor.matmul(
                                    pbig[s],
                                    lhsT=lhsT,
                                    rhs=adj_sb[:, mc, s * FS : (s + 1) * FS],
                                    start=(mc == 0),
                                    stop=(mc == NT - 1),
                                )
                            nc.tensor.matmul(
                                pbig[NS][:, : 1 + D],
                                lhsT=lhsT,
                                rhs=onesW[:, mc, : 1 + D],
                                start=(mc == 0),
                                stop=(mc == NT - 1),
                            )
                    rinv = stage.tile([P, 1], F32, tag="rinv", bufs=4, name=f"ri{nch}")
                    nc.vector.reciprocal(rinv, pbig[NS][:, 0:1])
                    fst = stage.tile([P, D], F32, tag="fst", bufs=3, name=f"fst{nch}")
                    nc.vector.tensor_scalar_mul(
                        out=fst, in0=pbig[NS][:, 1 : 1 + D], scalar1=rinv
                    )
                    nc.sync.dma_start(out=ofeat_d[:, nch, :], in_=fst)
                    ast = stage.tile([P, N], F32, tag="ast", bufs=2, name=f"ast{nch}")
                    for s in range(NS):
                        nc.vector.tensor_scalar_mul(
                            out=ast[:, s * FS : (s + 1) * FS],
                            in0=pbig[s],
                            scalar1=rinv,
                        )
                    nc.sync.dma_start(out=oadj_d[:, nch, :], in_=ast)

    nc.compile()
    return nc


def kernel(text, adj, Wq, bq, Wk, bk, Wv, bv):
    global LAST_RESULTS
    if "nc" not in _CACHE:
        _CACHE["nc"] = _build()
    nc = _CACHE["nc"]

    text = np.asarray(text, dtype=np.float32)
    adj = np.asarray(adj, dtype=np.float32)
    shared = {
        "Wq": np.asarray(Wq, dtype=np.float32),
        "bq": np.asarray(bq, dtype=np.float32),
        "Wk": np.asarray(Wk, dtype=np.float32),
        "bk": np.asarray(bk, dtype=np.float32),
        "Wv": np.asarray(Wv, dtype=np.float32),
        "bv": np.asarray(bv, dtype=np.float32),
    }
    B = text.shape[0]
    in_maps = [{"text": text[i], "adj": adj[i], **shared} for i in range(B)]
    res = run_bass_kernel_spmd(nc, in_maps, core_ids=list(range(B)))
    LAST_RESULTS = res
    output = np.stack([r["out_feat"] for r in res.results])
    new_adj = np.stack([r["out_adj"] for r in res.results])
    return output, new_adj
